# revision 41
# baseline (speedup 1.0000x reference)
# Trainium2 Bass kernel for multi-head attention (B=8, N=1024, C=768, H=12).
# Sharding: data-parallel over batch — one batch element per NeuronCore (8 cores).
#
# Per-core design:
#   - transposed activation layout ([feature, token]) so matmuls contract over
#     the partition dim; bf16 TensorEngine compute, fp32 accumulation
#   - inputs are pre-cast to bf16 on the host: no on-chip dtype conversion and
#     half the DMA bytes; x spreads over all three DMA queues and w_qkv's q/k
#     half loads per 128-column block (3D-AP DMAs) ordered by first use
#   - softmax without max-subtraction (scores ~ N(0,1)); denominators come from
#     a fused [v | 1] moving operand in the P^T@V matmul
#   - P@V is computed in [query, D] layout (lhsT = P^T chunk, rhs = [v | 1]):
#     all 128 output partitions are queries, halving PE cycles vs the [D+1,
#     token] layout; the denominator lands per-partition so normalization is a
#     DVE reciprocal + per-partition tensor_scalar multiply, no PE matmuls
#   - the normalized attention output ([token, feature]) is transposed back to
#     [feature, token] in 4-block batches (one DVE copy per 4 PE transposes)
#     in the PE holes of the ACT-paced late slots, then projected
#   - phase C is software-pipelined: head h's S matmuls (paced by the exp-bound
#     ACT engine) run in slot h, its U matmuls in slot h+2; all v matmuls ride
#     in slot 0 behind S(0) so the exp pipeline starts right after the input
#     phase; q/k projection chunks weave into the remaining PE idle
#   - reps are cross-pipelined: rep r's output projection (+ bias + out-DMA)
#     rides one token-block per slot inside rep r+1's ACT-paced slots 2-9
#     (through the spare qk psum bufs), and rep r+1's input DMAs + x
#     transposes + head-0 q/k projection are emitted in rep r's drain slots,
#     so the exp stream barely pauses between reps. w_proj is parity
#     double-buffered since the previous rep's projection reads it while the
#     next rep's load is in flight.
import numpy as np

B, N, C = 8, 1024, 768
H, D = 12, 64
SCALE = D ** -0.5
NCORES = 8
NRC = N // 128   # 8 row (token/key) chunks
NCC = C // 128   # 6 channel chunks

_cached_nc = {}
PHASE_MARKS = []


def _mark(nc, label):
    PHASE_MARKS.append((nc.next_id(), label))


def _build(reps=1):
    try:
        import concourse  # noqa: F401
    except ImportError:
        import sys
        sys.path.insert(0, "/opt/trn_rl_repo")
    import concourse.bass as bass
    import concourse.tile as tile
    from concourse import bacc, mybir
    from concourse.masks import make_identity

    f32 = mybir.dt.float32
    bf16 = mybir.dt.bfloat16
    EXP = mybir.ActivationFunctionType.Exp

    nc = bacc.Bacc("TRN2", target_bir_lowering=False, debug=False, num_devices=NCORES)
    x_d = nc.dram_tensor("x_bf", [N, C], bf16, kind="ExternalInput").ap()
    wqkv_d = nc.dram_tensor("wqkv_bf", [C, 3 * C], bf16, kind="ExternalInput").ap()
    wproj_d = nc.dram_tensor("wproj_bf", [C, C], bf16, kind="ExternalInput").ap()
    bproj_d = nc.dram_tensor("b_proj", [C], f32, kind="ExternalInput").ap()
    out_d = nc.dram_tensor("out", [N, C], f32, kind="ExternalOutput").ap()

    with tile.TileContext(nc) as tc:
        with (
            tc.tile_pool(name="persist", bufs=1) as persist,
            tc.tile_pool(name="stage", bufs=3) as stage,
            tc.tile_pool(name="small", bufs=2) as small,
            tc.tile_pool(name="pTp", bufs=24) as pTp,
            tc.tile_pool(name="ps", bufs=1, space="PSUM") as ps,
        ):
            # ---- constants (once) ----
            bias_t = persist.tile([128, C], f32, tag="bias_t")
            ident_bf = persist.tile([128, 128], bf16, tag="ident_bf")
            make_identity(nc, ident_bf)

            def emit_A_dma(r):
                """Input DMAs for rep r (x chunks + wv + wp; wq comes in
                emit_A_wq since its previous generation is read until slot
                11). Returns the tile handles rep r's compute will use."""
                T = {}
                T["wv"] = persist.tile([128, NCC, C], bf16, tag="wv_bf", name=f"wv_{r}")
                T["wp"] = persist.tile([128, NCC, C], bf16, tag=f"wp{r % 2}", name=f"wp_{r}")
                x_eng = [nc.sync, nc.gpsimd, nc.sync, nc.gpsimd,
                         nc.sync, nc.scalar, nc.sync, nc.scalar]
                xbs = [None] * NRC
                for rc in (0, 1, 2, 3, 4, 6, 5, 7):
                    xb = stage.tile([128, C], bf16, tag="xbf", bufs=NRC)
                    x_eng[rc].dma_start(out=xb, in_=x_d[rc * 128:(rc + 1) * 128, :])
                    xbs[rc] = xb
                T["xbs"] = xbs
                for cc in range(NCC):
                    sl_r = slice(cc * 128, (cc + 1) * 128)
                    nc.sync.dma_start(out=T["wv"][:, cc, :],
                                      in_=wqkv_d[sl_r, 2 * C:3 * C])
                for cc in range(NCC):
                    nc.scalar.dma_start(
                        out=T["wp"][:, cc, :], in_=wproj_d[cc * 128:(cc + 1) * 128, :]
                    )
                if r == 0:
                    nc.gpsimd.dma_start(
                        out=bias_t,
                        in_=bass.AP(
                            tensor=bproj_d.tensor, offset=bproj_d.offset,
                            ap=[[0, 128], [1, C]]
                        ),
                    )
                return T

            def emit_A_wq(T):
                # w_qkv q/k halves load per 128-column block spanning all 768
                # rows (one 3D-AP DMA each), ordered by first use: the head-0
                # q/k projection only needs blocks 0 and 6
                T["wq"] = persist.tile([128, 2 * NCC, NCC, 128], bf16, tag="wq_bf", name=f"wq_{id(T)}")
                for p in range(NCC):
                    for b in (p, NCC + p):
                        nc.gpsimd.dma_start(
                            out=T["wq"][:, b, :, :],
                            in_=bass.AP(
                                tensor=wqkv_d.tensor,
                                offset=wqkv_d.offset + b * 128,
                                ap=[[3 * C, 128], [128 * 3 * C, NCC], [1, 128]],
                            ),
                        )

            def emit_A_compute(T):
                """x transposes (nh-outer, 4 batched per psum tile + one DVE
                copy) chased by the head-0 q/k projection."""
                T["xT"] = persist.tile([128, NCC, N], bf16, tag="xT", name=f"xT_{id(T)}")
                T["qT"] = persist.tile([128, NCC, N], bf16, tag="qT", name=f"qT_{id(T)}")
                T["kT"] = persist.tile([128, NCC, N], bf16, tag="kT", name=f"kT_{id(T)}")
                xbs = T["xbs"]
                for nh in range(2):
                    sl = slice(nh * 512, (nh + 1) * 512)
                    q0 = ps.tile([128, 512], f32, tag="qk", bufs=2)
                    k0 = ps.tile([128, 512], f32, tag="qk", bufs=2)
                    for cc in range(NCC):
                        pt = ps.tile([128, 4, 128], bf16, tag="s", bufs=2)
                        for j, rc in enumerate(range(nh * 4, nh * 4 + 4)):
                            nc.tensor.transpose(
                                pt[:, j, :], xbs[rc][:, cc * 128:(cc + 1) * 128],
                                ident_bf
                            )
                        nc.vector.tensor_copy(out=T["xT"][:, cc, sl], in_=pt)
                        st = dict(start=(cc == 0), stop=(cc == NCC - 1))
                        for blk, dst in ((0, q0), (NCC, k0)):
                            nc.tensor.matmul(
                                dst, T["wq"][:, blk, cc, :], T["xT"][:, cc, sl], **st
                            )
                    nc.vector.tensor_copy(out=T["qT"][:, 0, sl], in_=q0)
                    nc.vector.tensor_copy(out=T["kT"][:, 0, sl], in_=k0)

            def emit_proj(prev, rc):
                """Output projection of the PREVIOUS rep for one token block:
                y = attn_out @ w_proj + bias, split in halves across both
                vector-add targets and both DMA queues."""
                attnT_p, wp_p = prev
                ya = ps.tile([128, 512], f32, tag="qk", bufs=2)
                yb = ps.tile([128, 256], f32, tag="qk", bufs=2)
                for cc in range(NCC):
                    lhsT = attnT_p[:, cc, rc * 128:(rc + 1) * 128]
                    st = dict(start=(cc == 0), stop=(cc == NCC - 1))
                    nc.tensor.matmul(ya, lhsT, wp_p[:, cc, 0:512], **st)
                    nc.tensor.matmul(yb, lhsT, wp_p[:, cc, 512:768], **st)
                ysb = small.tile([128, C], f32, tag="ysb")
                nc.vector.tensor_add(out=ysb[:, 0:512], in0=ya,
                                     in1=bias_t[:, 0:512])
                nc.sync.dma_start(out=out_d[rc * 128:(rc + 1) * 128, 0:512],
                                  in_=ysb[:, 0:512])
                nc.vector.tensor_add(out=ysb[:, 512:768], in0=yb,
                                     in1=bias_t[:, 512:768])
                nc.scalar.dma_start(out=out_d[rc * 128:(rc + 1) * 128, 512:768],
                                    in_=ysb[:, 512:768])

            # ---- prologue: rep 0's inputs ----
            A_cur = emit_A_dma(0)
            emit_A_wq(A_cur)
            emit_A_compute(A_cur)
            prev = None  # (attnT, wp) of the rep whose projection is pending

            for _rep in range(reps):
                xT, qT, kT = A_cur["xT"], A_cur["qT"], A_cur["kT"]
                wq_bf, wv_bf, wp_bf = A_cur["wq"], A_cur["wv"], A_cur["wp"]
                attn_sb = persist.tile([128, NRC, C], bf16, tag="attn_sb")
                vaug = persist.tile([128, NRC, H, D + 1], bf16, tag="vaug")
                # attnT is parity double-buffered: the previous rep's woven
                # projection reads its buffer through slot 11 while this rep's
                # transposes fill the other one from slot 10
                attnT = persist.tile([128, NCC, N], bf16, tag=f"attnT{_rep % 2}",
                                     name=f"attnT_{_rep}")

                # ---------- emission helpers ----------
                def emit_qk_mms(state):
                    """Emit the next pending q/k-chunk matmul (one at a time)."""
                    if not state:
                        return
                    _due, mc, nh, cc, qp = state[0]
                    dst = qT if mc < NCC else kT
                    nc.tensor.matmul(
                        qp,
                        wq_bf[:, mc, cc, :],
                        xT[:, cc, nh * 512:(nh + 1) * 512],
                        start=(cc == 0),
                        stop=(cc == NCC - 1),
                    )
                    if cc == NCC - 1:
                        nc.vector.tensor_copy(
                            out=dst[:, mc % NCC, nh * 512:(nh + 1) * 512], in_=qp
                        )
                    state.pop(0)

                def queue_qk(mc, due):
                    st = []
                    for nh in range(2):
                        qp = ps.tile([128, 512], f32, tag="qk", bufs=2)
                        for cc in range(NCC):
                            st.append((due, mc, nh, cc, qp))
                    return st

                def emit_v(rc):
                    vpa = ps.tile([128, 512], f32, tag="qk", bufs=2)
                    vpb = ps.tile([128, 256], f32, tag="qk", bufs=2)
                    for cc in range(NCC):
                        lhsT = xT[:, cc, rc * 128:(rc + 1) * 128]
                        st = dict(start=(cc == 0), stop=(cc == NCC - 1))
                        nc.tensor.matmul(vpa, lhsT, wv_bf[:, cc, 0:512], **st)
                        nc.tensor.matmul(vpb, lhsT, wv_bf[:, cc, 512:768], **st)
                    nc.vector.tensor_copy(
                        out=vaug[:, rc, 0:8, 0:D],
                        in_=vpa.rearrange("p (a d) -> p a d", d=D),
                    )
                    nc.vector.tensor_copy(
                        out=vaug[:, rc, 8:12, 0:D],
                        in_=vpb.rearrange("p (a d) -> p a d", d=D),
                    )
                    nc.vector.memset(vaug[:, rc, :, D:D + 1], 1.0)

                def emit_S(h, kc, pT_tiles):
                    cc, off = h // 2, (h % 2) * 64
                    s_ps = ps.tile([128, N], f32, tag="s", bufs=2)
                    for nh in range(2):
                        sl = slice(nh * 512, (nh + 1) * 512)
                        nc.tensor.matmul(
                            s_ps[:, sl],
                            kT[off:off + 64, cc, kc * 128:(kc + 1) * 128],
                            qT[off:off + 64, cc, sl],
                            start=True,
                            stop=True,
                        )
                    pT_t = pTp.tile([128, N], bf16, tag="pT")
                    nc.scalar.activation(out=pT_t, in_=s_ps, func=EXP, scale=SCALE)
                    pT_tiles[kc] = pT_t

                def emit_UT(h, qb, pT_tiles):
                    # u[q, 0:64] = sum_k P[k,q] v[k,:], u[q,64] = softmax denom
                    u_ps = ps.tile([128, D + 1], f32, tag="u", bufs=2)
                    for kc in range(NRC):
                        nc.tensor.matmul(
                            u_ps,
                            pT_tiles[kc][:, qb * 128:(qb + 1) * 128],
                            vaug[:, kc, h, :],
                            start=(kc == 0),
                            stop=(kc == NRC - 1),
                        )
                    rcp = small.tile([128, 1], f32, tag="rcp", bufs=4)
                    nc.vector.reciprocal(out=rcp, in_=u_ps[:, D:D + 1])
                    # gpsimd/Pool cannot read PSUM, so this stays on DVE
                    nc.vector.tensor_scalar_mul(
                        out=attn_sb[:, qb, h * 64:(h + 1) * 64],
                        in0=u_ps[:, 0:D],
                        scalar1=rcp,
                    )

                def emit_tr(p):
                    # attn_sb [token, feature] -> attnT [feature, token];
                    # 4 row-chunks batched per psum tile + single DVE copy
                    for rc0 in (0, 4):
                        tp = ps.tile([128, 4, 128], bf16, tag="u", bufs=2)
                        for j in range(4):
                            nc.tensor.transpose(
                                tp[:, j, :],
                                attn_sb[:, rc0 + j, p * 128:(p + 1) * 128], ident_bf
                            )
                        nc.vector.tensor_copy(
                            out=attnT[:, p, rc0 * 128:(rc0 + 4) * 128], in_=tp
                        )

                # ---------- phase C: head-pipelined ----------
                # slot t: S-block of head t (t<H), U-block of head t-2 (t>=2),
                # previous rep's projection block rc=t-2 (slots 2-9)
                _mark(nc, f"C:rep{_rep}")
                qk_state = []
                pT_all = [dict() for _ in range(H)]
                for t in range(H + 2):
                    if t < H and t % 2 == 0 and t // 2 + 1 < NCC:
                        qk_state += queue_qk(t // 2 + 1, t + 2)
                    elif t < H and t % 2 == 1 and t // 2 + 1 < NCC:
                        qk_state += queue_qk(NCC + t // 2 + 1, t + 1)

                    # anything the S-block of head t reads must be complete
                    while qk_state and qk_state[0][0] <= t:
                        emit_qk_mms(qk_state)

                    for i in range(NRC):
                        if t < H:
                            emit_S(t, i, pT_all[t])
                        # all v matmuls ride in slot 0 behind S(0): the exp
                        # pipeline starts immediately after the input phase
                        # while the PE chews v (chasing the wv DMA arrivals)
                        if t == 0:
                            emit_v(i)
                        if t >= 1:
                            emit_qk_mms(qk_state)
                        if t >= 2:
                            emit_UT(t - 2, i, pT_all[t - 2])
                        if t >= 1:
                            emit_qk_mms(qk_state)
                    # the previous rep's projection rides slots 4-11: the
                    # early slots are already PE-rich with qk chunks while
                    # slots 9-11 would otherwise starve the PE
                    if 4 <= t <= 11 and prev is not None:
                        emit_proj(prev, t - 4)
                    if t == 10:
                        emit_tr(0)
                        emit_tr(1)
                        # next rep's x / wv / wp loads start here: their
                        # previous generations have no readers past slot 1
                        if _rep + 1 < reps:
                            A_next = emit_A_dma(_rep + 1)
                    elif t == 11:
                        emit_tr(2)
                        emit_tr(3)
                    elif t == 12:
                        emit_tr(4)
                        # wq reload + next rep's transposes and head-0 q/k
                        # projection fill the ACT-idle drain slots (the last
                        # qk-chunk readers of wq/xT finished in slot 11)
                        if _rep + 1 < reps:
                            emit_A_wq(A_next)
                            emit_A_compute(A_next)
                    elif t == 13:
                        emit_tr(5)

                prev = (attnT, wp_bf)
                if _rep + 1 < reps:
                    A_cur = A_next

            # ---- tail: last rep's projection ----
            _mark(nc, "D:final")
            for rc in range(NRC):
                emit_proj(prev, rc)

    nc.compile()
    return nc


def _get_nc(reps=1):
    if reps not in _cached_nc:
        _cached_nc[reps] = _build(reps)
    return _cached_nc[reps]


def _to_bf16(a):
    import ml_dtypes
    return np.asarray(a, dtype=np.float32).astype(ml_dtypes.bfloat16)


def _in_maps(x, w_qkv, w_proj, b_proj):
    wq = _to_bf16(w_qkv)
    wp = _to_bf16(w_proj)
    bp = np.asarray(b_proj, dtype=np.float32)
    return [
        {
            "x_bf": _to_bf16(np.asarray(x)[b]),
            "wqkv_bf": wq,
            "wproj_bf": wp,
            "b_proj": bp,
        }
        for b in range(NCORES)
    ]


def _run(nc, x, w_qkv, w_proj, b_proj):
    from concourse.bass_utils import run_bass_kernel_spmd

    in_maps = _in_maps(x, w_qkv, w_proj, b_proj)
    res = run_bass_kernel_spmd(nc, in_maps, core_ids=list(range(NCORES)))
    return np.stack([res.results[b]["out"] for b in range(NCORES)], axis=0)


def kernel(x, w_qkv, w_proj, b_proj):
    try:
        import concourse  # noqa: F401
    except ImportError:
        import sys
        sys.path.insert(0, "/opt/trn_rl_repo")

    return _run(_get_nc(1), x, w_qkv, w_proj, b_proj)


# revision 43
# speedup vs baseline: 1.2622x; 1.2622x over previous
# Trainium2 Bass kernel for multi-head attention (B=8, N=1024, C=768, H=12).
# Sharding: data-parallel over batch — one batch element per NeuronCore (8 cores).
#
# Per-core design:
#   - transposed activation layout ([feature, token]) so matmuls contract over
#     the partition dim; bf16 TensorEngine compute, fp32 accumulation
#   - inputs are pre-cast to bf16 on the host: no on-chip dtype conversion and
#     half the DMA bytes; x spreads over all three DMA queues and w_qkv's q/k
#     half loads per 128-column block (3D-AP DMAs) ordered by first use
#   - softmax without max-subtraction (scores ~ N(0,1)); denominators come from
#     a fused [v | 1] moving operand in the P^T@V matmul
#   - P@V is computed in [query, D] layout (lhsT = P^T chunk, rhs = [v | 1]):
#     all 128 output partitions are queries, halving PE cycles vs the [D+1,
#     token] layout; the denominator lands per-partition so normalization is a
#     DVE reciprocal + per-partition tensor_scalar multiply, no PE matmuls
#   - the normalized attention output ([token, feature]) is transposed back to
#     [feature, token] in 4-block batches (one DVE copy per 4 PE transposes)
#     in the PE holes of the ACT-paced late slots, then projected
#   - phase C is software-pipelined: head h's S matmuls (paced by the exp-bound
#     ACT engine) run in slot h, its U matmuls in slot h+2; all v matmuls ride
#     in slot 0 behind S(0) so the exp pipeline starts right after the input
#     phase; q/k projection chunks weave into the remaining PE idle
#   - reps are cross-pipelined: rep r's output projection (+ bias + out-DMA)
#     rides one token-block per slot inside rep r+1's slots 4-11 (through the
#     spare qk psum bufs) where the PE would otherwise starve as qk filler
#     work runs out, and rep r+1's input DMAs (slot 9), wq reload (slot 11)
#     and x transposes + head-0 q/k projection (slot 12) fill rep r's drain
#     slots, so the exp stream barely pauses between reps. w_proj and attnT
#     are parity double-buffered since the previous rep's projection reads
#     them while the next rep overwrites.
import numpy as np

B, N, C = 8, 1024, 768
H, D = 12, 64
SCALE = D ** -0.5
NCORES = 8
NRC = N // 128   # 8 row (token/key) chunks
NCC = C // 128   # 6 channel chunks

_cached_nc = {}
PHASE_MARKS = []


def _mark(nc, label):
    PHASE_MARKS.append((nc.next_id(), label))


def _build(reps=1):
    try:
        import concourse  # noqa: F401
    except ImportError:
        import sys
        sys.path.insert(0, "/opt/trn_rl_repo")
    import concourse.bass as bass
    import concourse.tile as tile
    from concourse import bacc, mybir
    from concourse.masks import make_identity

    f32 = mybir.dt.float32
    bf16 = mybir.dt.bfloat16
    EXP = mybir.ActivationFunctionType.Exp

    nc = bacc.Bacc("TRN2", target_bir_lowering=False, debug=False, num_devices=NCORES)
    x_d = nc.dram_tensor("x_bf", [N, C], bf16, kind="ExternalInput").ap()
    wqkv_d = nc.dram_tensor("wqkv_bf", [C, 3 * C], bf16, kind="ExternalInput").ap()
    wproj_d = nc.dram_tensor("wproj_bf", [C, C], bf16, kind="ExternalInput").ap()
    bproj_d = nc.dram_tensor("b_proj", [C], f32, kind="ExternalInput").ap()
    out_d = nc.dram_tensor("out", [N, C], f32, kind="ExternalOutput").ap()

    with tile.TileContext(nc) as tc:
        with (
            tc.tile_pool(name="persist", bufs=1) as persist,
            tc.tile_pool(name="stage", bufs=3) as stage,
            tc.tile_pool(name="small", bufs=2) as small,
            tc.tile_pool(name="pTp", bufs=24) as pTp,
            tc.tile_pool(name="ps", bufs=1, space="PSUM") as ps,
        ):
            # ---- constants (once) ----
            bias_t = persist.tile([128, C], f32, tag="bias_t")
            ident_bf = persist.tile([128, 128], bf16, tag="ident_bf")
            make_identity(nc, ident_bf)

            def emit_A_dma(r):
                """Input DMAs for rep r (x chunks + wv + wp; wq comes in
                emit_A_wq since its previous generation is read until slot
                11). Returns the tile handles rep r's compute will use."""
                T = {}
                T["wv"] = persist.tile([128, NCC, C], bf16, tag="wv_bf", name=f"wv_{r}")
                T["wp"] = persist.tile([128, NCC, C], bf16, tag=f"wp{r % 2}", name=f"wp_{r}")
                x_eng = [nc.sync, nc.gpsimd, nc.sync, nc.gpsimd,
                         nc.sync, nc.scalar, nc.sync, nc.scalar]
                xbs = [None] * NRC
                for rc in (0, 1, 2, 3, 4, 6, 5, 7):
                    xb = stage.tile([128, C], bf16, tag="xbf", bufs=NRC)
                    x_eng[rc].dma_start(out=xb, in_=x_d[rc * 128:(rc + 1) * 128, :])
                    xbs[rc] = xb
                T["xbs"] = xbs
                for cc in range(NCC):
                    sl_r = slice(cc * 128, (cc + 1) * 128)
                    nc.sync.dma_start(out=T["wv"][:, cc, :],
                                      in_=wqkv_d[sl_r, 2 * C:3 * C])
                for cc in range(NCC):
                    nc.scalar.dma_start(
                        out=T["wp"][:, cc, :], in_=wproj_d[cc * 128:(cc + 1) * 128, :]
                    )
                if r == 0:
                    nc.gpsimd.dma_start(
                        out=bias_t,
                        in_=bass.AP(
                            tensor=bproj_d.tensor, offset=bproj_d.offset,
                            ap=[[0, 128], [1, C]]
                        ),
                    )
                return T

            def emit_A_wq(T):
                # w_qkv q/k halves load per 128-column block spanning all 768
                # rows (one 3D-AP DMA each), ordered by first use: the head-0
                # q/k projection only needs blocks 0 and 6
                T["wq"] = persist.tile([128, 2 * NCC, NCC, 128], bf16, tag="wq_bf", name=f"wq_{id(T)}")
                for p in range(NCC):
                    for b in (p, NCC + p):
                        nc.gpsimd.dma_start(
                            out=T["wq"][:, b, :, :],
                            in_=bass.AP(
                                tensor=wqkv_d.tensor,
                                offset=wqkv_d.offset + b * 128,
                                ap=[[3 * C, 128], [128 * 3 * C, NCC], [1, 128]],
                            ),
                        )

            def emit_A_compute(T):
                """x transposes (nh-outer, 4 batched per psum tile + one DVE
                copy) chased by the head-0 q/k projection."""
                T["xT"] = persist.tile([128, NCC, N], bf16, tag="xT", name=f"xT_{id(T)}")
                T["qT"] = persist.tile([128, NCC, N], bf16, tag="qT", name=f"qT_{id(T)}")
                T["kT"] = persist.tile([128, NCC, N], bf16, tag="kT", name=f"kT_{id(T)}")
                xbs = T["xbs"]
                for nh in range(2):
                    sl = slice(nh * 512, (nh + 1) * 512)
                    q0 = ps.tile([128, 512], f32, tag="qk", bufs=2)
                    k0 = ps.tile([128, 512], f32, tag="qk", bufs=2)
                    for cc in range(NCC):
                        pt = ps.tile([128, 4, 128], bf16, tag="s", bufs=2)
                        for j, rc in enumerate(range(nh * 4, nh * 4 + 4)):
                            nc.tensor.transpose(
                                pt[:, j, :], xbs[rc][:, cc * 128:(cc + 1) * 128],
                                ident_bf
                            )
                        nc.vector.tensor_copy(out=T["xT"][:, cc, sl], in_=pt)
                        st = dict(start=(cc == 0), stop=(cc == NCC - 1))
                        for blk, dst in ((0, q0), (NCC, k0)):
                            nc.tensor.matmul(
                                dst, T["wq"][:, blk, cc, :], T["xT"][:, cc, sl], **st
                            )
                    nc.vector.tensor_copy(out=T["qT"][:, 0, sl], in_=q0)
                    nc.vector.tensor_copy(out=T["kT"][:, 0, sl], in_=k0)

            def emit_proj(prev, rc):
                """Output projection of the PREVIOUS rep for one token block:
                y = attn_out @ w_proj + bias, split in halves across both
                vector-add targets and both DMA queues."""
                attnT_p, wp_p = prev
                ya = ps.tile([128, 512], f32, tag="qk", bufs=2)
                yb = ps.tile([128, 256], f32, tag="qk", bufs=2)
                for cc in range(NCC):
                    lhsT = attnT_p[:, cc, rc * 128:(rc + 1) * 128]
                    st = dict(start=(cc == 0), stop=(cc == NCC - 1))
                    nc.tensor.matmul(ya, lhsT, wp_p[:, cc, 0:512], **st)
                    nc.tensor.matmul(yb, lhsT, wp_p[:, cc, 512:768], **st)
                ysb = small.tile([128, C], f32, tag="ysb")
                nc.vector.tensor_add(out=ysb[:, 0:512], in0=ya,
                                     in1=bias_t[:, 0:512])
                nc.sync.dma_start(out=out_d[rc * 128:(rc + 1) * 128, 0:512],
                                  in_=ysb[:, 0:512])
                nc.vector.tensor_add(out=ysb[:, 512:768], in0=yb,
                                     in1=bias_t[:, 512:768])
                nc.scalar.dma_start(out=out_d[rc * 128:(rc + 1) * 128, 512:768],
                                    in_=ysb[:, 512:768])

            # ---- prologue: rep 0's inputs ----
            A_cur = emit_A_dma(0)
            emit_A_wq(A_cur)
            emit_A_compute(A_cur)
            prev = None  # (attnT, wp) of the rep whose projection is pending

            for _rep in range(reps):
                xT, qT, kT = A_cur["xT"], A_cur["qT"], A_cur["kT"]
                wq_bf, wv_bf, wp_bf = A_cur["wq"], A_cur["wv"], A_cur["wp"]
                attn_sb = persist.tile([128, NRC, C], bf16, tag="attn_sb")
                vaug = persist.tile([128, NRC, H, D + 1], bf16, tag="vaug")
                # attnT is parity double-buffered: the previous rep's woven
                # projection reads its buffer through slot 11 while this rep's
                # transposes fill the other one from slot 10
                attnT = persist.tile([128, NCC, N], bf16, tag=f"attnT{_rep % 2}",
                                     name=f"attnT_{_rep}")

                # ---------- emission helpers ----------
                def emit_qk_mms(state):
                    """Emit the next pending q/k-chunk matmul (one at a time)."""
                    if not state:
                        return
                    _due, mc, nh, cc, qp = state[0]
                    dst = qT if mc < NCC else kT
                    nc.tensor.matmul(
                        qp,
                        wq_bf[:, mc, cc, :],
                        xT[:, cc, nh * 512:(nh + 1) * 512],
                        start=(cc == 0),
                        stop=(cc == NCC - 1),
                    )
                    if cc == NCC - 1:
                        nc.vector.tensor_copy(
                            out=dst[:, mc % NCC, nh * 512:(nh + 1) * 512], in_=qp
                        )
                    state.pop(0)

                def queue_qk(mc, due):
                    st = []
                    for nh in range(2):
                        qp = ps.tile([128, 512], f32, tag="qk", bufs=2)
                        for cc in range(NCC):
                            st.append((due, mc, nh, cc, qp))
                    return st

                def emit_v(rc):
                    vpa = ps.tile([128, 512], f32, tag="qk", bufs=2)
                    vpb = ps.tile([128, 256], f32, tag="qk", bufs=2)
                    for cc in range(NCC):
                        lhsT = xT[:, cc, rc * 128:(rc + 1) * 128]
                        st = dict(start=(cc == 0), stop=(cc == NCC - 1))
                        nc.tensor.matmul(vpa, lhsT, wv_bf[:, cc, 0:512], **st)
                        nc.tensor.matmul(vpb, lhsT, wv_bf[:, cc, 512:768], **st)
                    nc.vector.tensor_copy(
                        out=vaug[:, rc, 0:8, 0:D],
                        in_=vpa.rearrange("p (a d) -> p a d", d=D),
                    )
                    nc.vector.tensor_copy(
                        out=vaug[:, rc, 8:12, 0:D],
                        in_=vpb.rearrange("p (a d) -> p a d", d=D),
                    )
                    nc.vector.memset(vaug[:, rc, :, D:D + 1], 1.0)

                def emit_S(h, kc, pT_tiles):
                    cc, off = h // 2, (h % 2) * 64
                    s_ps = ps.tile([128, N], f32, tag="s", bufs=2)
                    for nh in range(2):
                        sl = slice(nh * 512, (nh + 1) * 512)
                        nc.tensor.matmul(
                            s_ps[:, sl],
                            kT[off:off + 64, cc, kc * 128:(kc + 1) * 128],
                            qT[off:off + 64, cc, sl],
                            start=True,
                            stop=True,
                        )
                    pT_t = pTp.tile([128, N], bf16, tag="pT")
                    nc.scalar.activation(out=pT_t, in_=s_ps, func=EXP, scale=SCALE)
                    pT_tiles[kc] = pT_t

                def emit_UT(h, qb, pT_tiles):
                    # u[q, 0:64] = sum_k P[k,q] v[k,:], u[q,64] = softmax denom
                    u_ps = ps.tile([128, D + 1], f32, tag="u", bufs=2)
                    for kc in range(NRC):
                        nc.tensor.matmul(
                            u_ps,
                            pT_tiles[kc][:, qb * 128:(qb + 1) * 128],
                            vaug[:, kc, h, :],
                            start=(kc == 0),
                            stop=(kc == NRC - 1),
                        )
                    rcp = small.tile([128, 1], f32, tag="rcp", bufs=4)
                    nc.vector.reciprocal(out=rcp, in_=u_ps[:, D:D + 1])
                    # gpsimd/Pool cannot read PSUM, so this stays on DVE
                    nc.vector.tensor_scalar_mul(
                        out=attn_sb[:, qb, h * 64:(h + 1) * 64],
                        in0=u_ps[:, 0:D],
                        scalar1=rcp,
                    )

                def emit_tr(p):
                    # attn_sb [token, feature] -> attnT [feature, token];
                    # 4 row-chunks batched per psum tile + single DVE copy
                    for rc0 in (0, 4):
                        tp = ps.tile([128, 4, 128], bf16, tag="u", bufs=2)
                        for j in range(4):
                            nc.tensor.transpose(
                                tp[:, j, :],
                                attn_sb[:, rc0 + j, p * 128:(p + 1) * 128], ident_bf
                            )
                        nc.vector.tensor_copy(
                            out=attnT[:, p, rc0 * 128:(rc0 + 4) * 128], in_=tp
                        )

                # ---------- phase C: head-pipelined ----------
                # slot t: S-block of head t (t<H), U-block of head t-2 (t>=2),
                # previous rep's projection block rc=t-2 (slots 2-9)
                _mark(nc, f"C:rep{_rep}")
                qk_state = []
                pT_all = [dict() for _ in range(H)]
                for t in range(H + 2):
                    if t < H and t % 2 == 0 and t // 2 + 1 < NCC:
                        qk_state += queue_qk(t // 2 + 1, t + 2)
                    elif t < H and t % 2 == 1 and t // 2 + 1 < NCC:
                        qk_state += queue_qk(NCC + t // 2 + 1, t + 1)

                    # anything the S-block of head t reads must be complete
                    while qk_state and qk_state[0][0] <= t:
                        emit_qk_mms(qk_state)

                    for i in range(NRC):
                        if t < H:
                            emit_S(t, i, pT_all[t])
                        # all v matmuls ride in slot 0 behind S(0): the exp
                        # pipeline starts immediately after the input phase
                        # while the PE chews v (chasing the wv DMA arrivals)
                        if t == 0:
                            emit_v(i)
                        if t >= 1:
                            emit_qk_mms(qk_state)
                        if t >= 2:
                            emit_UT(t - 2, i, pT_all[t - 2])
                        if t >= 1:
                            emit_qk_mms(qk_state)
                    # the previous rep's projection rides slots 4-11: the
                    # early slots are already PE-rich with qk chunks while
                    # slots 9-11 would otherwise starve the PE
                    if 4 <= t <= 11 and prev is not None:
                        emit_proj(prev, t - 4)
                    if t == 9:
                        # next rep's x / wv / wp loads start here: their
                        # previous generations have no readers past slot 1,
                        # and issuing a slot early gives the transfers a full
                        # slot of headroom before the drain-slot compute
                        if _rep + 1 < reps:
                            A_next = emit_A_dma(_rep + 1)
                    elif t == 10:
                        emit_tr(0)
                        emit_tr(1)
                    elif t == 11:
                        emit_tr(2)
                        emit_tr(3)
                        # the last qk-chunk matmul reading the old wq was
                        # emitted in slot 10, so the reload is safe here and
                        # blocks 0/6 land before the slot-12 q/k projection
                        if _rep + 1 < reps:
                            emit_A_wq(A_next)
                    elif t == 12:
                        emit_tr(4)
                        if _rep + 1 < reps:
                            emit_A_compute(A_next)
                    elif t == 13:
                        emit_tr(5)

                prev = (attnT, wp_bf)
                if _rep + 1 < reps:
                    A_cur = A_next

            # ---- tail: last rep's projection ----
            _mark(nc, "D:final")
            for rc in range(NRC):
                emit_proj(prev, rc)

    nc.compile()
    return nc


def _get_nc(reps=1):
    if reps not in _cached_nc:
        _cached_nc[reps] = _build(reps)
    return _cached_nc[reps]


def _to_bf16(a):
    import ml_dtypes
    return np.asarray(a, dtype=np.float32).astype(ml_dtypes.bfloat16)


def _in_maps(x, w_qkv, w_proj, b_proj):
    wq = _to_bf16(w_qkv)
    wp = _to_bf16(w_proj)
    bp = np.asarray(b_proj, dtype=np.float32)
    return [
        {
            "x_bf": _to_bf16(np.asarray(x)[b]),
            "wqkv_bf": wq,
            "wproj_bf": wp,
            "b_proj": bp,
        }
        for b in range(NCORES)
    ]


def _run(nc, x, w_qkv, w_proj, b_proj):
    from concourse.bass_utils import run_bass_kernel_spmd

    in_maps = _in_maps(x, w_qkv, w_proj, b_proj)
    res = run_bass_kernel_spmd(nc, in_maps, core_ids=list(range(NCORES)))
    return np.stack([res.results[b]["out"] for b in range(NCORES)], axis=0)


def kernel(x, w_qkv, w_proj, b_proj):
    try:
        import concourse  # noqa: F401
    except ImportError:
        import sys
        sys.path.insert(0, "/opt/trn_rl_repo")

    return _run(_get_nc(1), x, w_qkv, w_proj, b_proj)
